# revision 6
# baseline (speedup 1.0000x reference)
"""AdaBlock (binarized double-conv residual block) Trainium2 kernel.

Strategy
--------
Data-parallel over batch: 16 images across 8 NeuronCores (2 images/core), no
collectives.  The binarized convs become exact +-1 matmuls: 3x3 conv = 9
shifted [Cin x spatial] matmuls accumulated in PSUM; per-out-channel scales
(ka * mean|w| * kw) are applied while draining PSUM.  Sign activations /
weights are exact in bf16, PSUM accumulation is fp32, so results match the
fp32 reference to ~1e-6.

Per-core pipeline (per image):
  DMA x -> s1 = sign(x + bias1_)       (bf16, spatially padded 66x66 layout)
  conv1: 2 outgrps x 8 row-blocks x (9 taps x 2 cin-halves) matmuls -> PSUM
  xres  = psum * sc1 + x               (in-place in the x buffer)
  s2    = sign(xres + bias2_)
  conv2: 8 row-blocks x 18 matmuls -> PSUM
  u     = psum * sc2 + bias3 + xres[:128];  u = prelu(u);  (+bias4)
  pixel-unshuffle via 4 strided copies fused with +bias4 -> DMA out
"""

import numpy as np
import ml_dtypes

import concourse.bass as bass
import concourse.mybir as mybir
from concourse import bacc
from concourse.tile import TileContext
from concourse.bass_utils import run_bass_kernel_spmd

B, C, H, W = 16, 256, 64, 64
NCORES = 8
BL = B // NCORES          # images per core
HW_ = H * W               # 4096
PH, PW = H + 2, W + 2     # 66, 66 padded
PS = PH * PW              # 4356
F32 = mybir.dt.float32
BF16 = mybir.dt.bfloat16

_CACHE = {}


def build_nc(reps=1):
    nc = bacc.Bacc()
    x_ext = nc.declare_dram_parameter("x", [BL, C, H, W], F32, isOutput=False)
    w1_ext = nc.declare_dram_parameter("w1", [128, 36, 128], BF16, isOutput=False)
    w2_ext = nc.declare_dram_parameter("w2", [128, 18, 128], BF16, isOutput=False)
    coef_ext = nc.declare_dram_parameter("coef", [128, 10], F32, isOutput=False)
    out_ext = nc.declare_dram_parameter("out", [BL, 2 * C, H // 2, W // 2], F32,
                                        isOutput=True)

    Ident = mybir.ActivationFunctionType.Identity
    Alu = mybir.AluOpType

    with TileContext(nc) as tc:
        with (
            tc.tile_pool(name="weights", bufs=1) as pw,
            tc.tile_pool(name="xbuf", bufs=4) as px,
            tc.tile_pool(name="signs", bufs=3) as psn,
            tc.tile_pool(name="small", bufs=4) as pt,
            tc.tile_pool(name="psA", bufs=4, space="PSUM") as psum1,
            tc.tile_pool(name="psB", bufs=4, space="PSUM") as psum2,
        ):
            w1_t = pw.tile([128, 36 * 128], BF16, tag="w1")
            nc.sync.dma_start(out=w1_t[:, :],
                              in_=w1_ext[:, :, :].rearrange("i a o -> i (a o)"))
            w2_t = pw.tile([128, 18 * 128], BF16, tag="w2")
            nc.sync.dma_start(out=w2_t[:, :],
                              in_=w2_ext[:, :, :].rearrange("i a o -> i (a o)"))
            coef_t = pw.tile([128, 10], F32, tag="coef")
            nc.sync.dma_start(out=coef_t[:, :], in_=coef_ext[:, :])

            st = [dict() for _ in range(BL)]

            def sign_stage(i, src_tiles, bias_col, tag):
                s = psn.tile([128, 2 * PS], BF16, tag="s")
                sv = s[:, :].rearrange("p (h y x) -> p h y x", h=2, y=PH, x=PW)
                for h in range(2):
                    nc.vector.memset(sv[:, h, 0, :], 0)
                    nc.vector.memset(sv[:, h, PH - 1, :], 0)
                    nc.vector.memset(sv[:, h, 1:PH - 1, 0], 0)
                    nc.vector.memset(sv[:, h, 1:PH - 1, PW - 1], 0)
                    nc.scalar.sign(
                        sv[:, h, 1:1 + H, 1:1 + W],
                        src_tiles[h][:, :].rearrange("p (y x) -> p y x", y=H, x=W),
                        bias=coef_t[:, bias_col + h:bias_col + h + 1],
                    )
                st[i][tag] = sv

            def stage_A(i):
                xs = []
                for h in range(2):
                    xt = px.tile([128, HW_], F32, tag="x")
                    nc.sync.dma_start(
                        out=xt[:, :],
                        in_=x_ext[i, h * 128:(h + 1) * 128, :, :].rearrange(
                            "c y x -> c (y x)"),
                    )
                    xs.append(xt)
                st[i]["x"] = xs
                sign_stage(i, xs, 2, "s1")

            def stage_B(i):  # conv1 + xres (in place into x)
                sv = st[i]["s1"]
                xs = st[i]["x"]
                for g in range(2):
                    for blk in range(8):
                        pt1 = psum1.tile([128, 512], F32, tag="ps1")
                        y0 = blk * 8
                        n = 0
                        for t in range(9):
                            ky, kx = t // 3, t % 3
                            for h in range(2):
                                idx = (g * 2 + h) * 9 + t
                                nc.tensor.matmul(
                                    pt1[:, :],
                                    w1_t[:, idx * 128:(idx + 1) * 128],
                                    sv[:, h, y0 + ky:y0 + ky + 8, kx:kx + W],
                                    start=(n == 0), stop=(n == 17),
                                )
                                n += 1
                        t1 = pt.tile([128, 512], F32, tag="t1")
                        nc.scalar.mul(t1[:, :], pt1[:, :], coef_t[:, g:g + 1])
                        xg = xs[g][:, blk * 512:(blk + 1) * 512]
                        nc.vector.tensor_add(xg, xg, t1[:, :])

            def stage_C(i):
                sign_stage(i, st[i]["x"], 4, "s2")

            def stage_D(i):  # conv2 + epilogue + out DMA
                sv = st[i]["s2"]
                x0 = st[i]["x"][0]
                for blk in range(8):
                    pt2 = psum2.tile([128, 512], F32, tag="ps2")
                    y0 = blk * 8
                    n = 0
                    for t in range(9):
                        ky, kx = t // 3, t % 3
                        for h in range(2):
                            idx = h * 9 + t
                            nc.tensor.matmul(
                                pt2[:, :],
                                w2_t[:, idx * 128:(idx + 1) * 128],
                                sv[:, h, y0 + ky:y0 + ky + 8, kx:kx + W],
                                start=(n == 0), stop=(n == 17),
                            )
                            n += 1
                    t2 = pt.tile([128, 512], F32, tag="t1")
                    nc.scalar.activation(t2[:, :], pt2[:, :], Ident,
                                         bias=coef_t[:, 7:8],
                                         scale=coef_t[:, 6:7])
                    xb = x0[:, blk * 512:(blk + 1) * 512]
                    nc.vector.tensor_add(xb, xb, t2[:, :])      # u = t2 + xres
                    m = pt.tile([128, 512], F32, tag="m")
                    nc.vector.tensor_scalar(m[:, :], xb, 0.0, coef_t[:, 8:9],
                                            op0=Alu.min, op1=Alu.mult)
                    nc.vector.tensor_add(xb, xb, m[:, :])       # u = prelu(u)
                uv = x0[:, :].rearrange("p (h2 r1 w2 r2) -> p r1 r2 h2 w2",
                                        h2=32, r1=2, w2=32, r2=2)
                od = out_ext[i, :, :, :].rearrange("(c j) y x -> c j y x", j=4)
                for j in range(4):
                    r1, r2 = j >> 1, j & 1
                    y = pt.tile([128, 1024], F32, tag="y")
                    nc.scalar.activation(
                        y[:, :].rearrange("p (a b) -> p a b", a=32, b=32),
                        uv[:, r1, r2, :, :], Ident,
                        bias=coef_t[:, 9:10], scale=1.0)
                    nc.sync.dma_start(
                        out=od[:, j, :, :],
                        in_=y[:, :].rearrange("p (a b) -> p a b", a=32, b=32))

            # software-pipelined emission: keep the PE busy across images
            for _ in range(reps):
                stage_A(0)
                stage_A(1)
                stage_B(0)
                stage_C(0)
                stage_B(1)
                stage_C(1)
                stage_D(0)
                stage_D(1)

    nc.compile()
    return nc


def _prep_weights(inputs):
    w1 = np.asarray(inputs["conv1_w"], np.float32)          # [256,256,3,3]
    w2 = np.asarray(inputs["conv2_w"], np.float32)          # [128,256,3,3]
    sc1 = (np.abs(w1).mean(axis=(1, 2, 3))
           * float(np.asarray(inputs["kw1"]))
           * float(np.asarray(inputs["ka1"]))).astype(np.float32)   # [256]
    sc2 = (np.abs(w2).mean(axis=(1, 2, 3))
           * float(np.asarray(inputs["kw2"]))
           * float(np.asarray(inputs["ka2"]))).astype(np.float32)   # [128]

    sgn1 = np.sign(w1).reshape(2, 128, 2, 128, 9)           # [g,o,h,i,t]
    w1b = np.ascontiguousarray(sgn1.transpose(3, 0, 2, 4, 1)
                               ).reshape(128, 36, 128).astype(ml_dtypes.bfloat16)
    sgn2 = np.sign(w2).reshape(128, 2, 128, 9)              # [o,h,i,t]
    w2b = np.ascontiguousarray(sgn2.transpose(2, 1, 3, 0)
                               ).reshape(128, 18, 128).astype(ml_dtypes.bfloat16)

    coef = np.zeros((128, 10), np.float32)
    coef[:, 0] = sc1[:128]
    coef[:, 1] = sc1[128:]
    b1 = np.asarray(inputs["bias1_"], np.float32).reshape(C)
    coef[:, 2] = b1[:128]
    coef[:, 3] = b1[128:]
    b2 = np.asarray(inputs["bias2_"], np.float32).reshape(C)
    coef[:, 4] = b2[:128]
    coef[:, 5] = b2[128:]
    coef[:, 6] = sc2
    coef[:, 7] = np.asarray(inputs["bias3"], np.float32).reshape(C // 2)
    coef[:, 8] = np.asarray(inputs["prelu2_w"], np.float32) - 1.0
    coef[:, 9] = np.asarray(inputs["bias4"], np.float32).reshape(C // 2)
    return w1b, w2b, coef


def kernel(**inputs):
    return kernel_with_results(**inputs)[0]


def kernel_with_results(trace=False, **inputs):
    x = np.ascontiguousarray(np.asarray(inputs["x"], np.float32))
    w1b, w2b, coef = _prep_weights(inputs)

    if "nc" not in _CACHE:
        _CACHE["nc"] = build_nc()
    nc = _CACHE["nc"]

    in_maps = [
        {"x": x[i * BL:(i + 1) * BL], "w1": w1b, "w2": w2b, "coef": coef}
        for i in range(NCORES)
    ]
    res = run_bass_kernel_spmd(nc, in_maps, core_ids=list(range(NCORES)),
                               trace=trace)
    out = np.concatenate([res.results[i]["out"] for i in range(NCORES)], axis=0)
    return out, res


# revision 9
# speedup vs baseline: 2.1785x; 2.1785x over previous
"""AdaBlock (binarized double-conv residual block) Trainium2 kernel.

Strategy
--------
Data-parallel over batch: 16 images across 8 NeuronCores (2 images/core), no
collectives.  The binarized convs become exact +-1 matmuls: 3x3 conv = 9
shifted [Cin x spatial] matmuls accumulated in PSUM; per-out-channel scales
(ka * mean|w| * kw) are applied while draining PSUM.  Sign activations /
weights are exact in bf16, PSUM accumulation is fp32, so results match the
fp32 reference to ~1e-6.

Per-core pipeline (per image):
  DMA x -> s1 = sign(x + bias1_)       (bf16, spatially padded 66x66 layout)
  conv1: 2 outgrps x 8 row-blocks x (9 taps x 2 cin-halves) matmuls -> PSUM
  xres  = psum * sc1 + x               (in-place in the x buffer)
  s2    = sign(xres + bias2_)
  conv2: 8 row-blocks x 18 matmuls -> PSUM
  u     = psum * sc2 + bias3 + xres[:128];  u = prelu(u);  (+bias4)
  pixel-unshuffle via 4 strided copies fused with +bias4 -> DMA out
"""

import numpy as np
import ml_dtypes

import concourse.bass as bass
import concourse.mybir as mybir
from concourse import bacc
from concourse.tile import TileContext
from concourse.bass_utils import run_bass_kernel_spmd

B, C, H, W = 16, 256, 64, 64
NCORES = 8
BL = B // NCORES          # images per core
HW_ = H * W               # 4096
PH, PW = H + 2, W + 2     # 66, 66 padded
PS = PH * PW              # 4356
F32 = mybir.dt.float32
BF16 = mybir.dt.bfloat16

_CACHE = {}


def build_nc(reps=1):
    nc = bacc.Bacc()
    x_ext = nc.declare_dram_parameter("x", [BL, C, H, W], F32, isOutput=False)
    w1_ext = nc.declare_dram_parameter("w1", [128, 36, 128], BF16, isOutput=False)
    w2_ext = nc.declare_dram_parameter("w2", [128, 18, 128], BF16, isOutput=False)
    coef_ext = nc.declare_dram_parameter("coef", [128, 10], F32, isOutput=False)
    out_ext = nc.declare_dram_parameter("out", [BL, 2 * C, H // 2, W // 2], F32,
                                        isOutput=True)

    Ident = mybir.ActivationFunctionType.Identity
    Alu = mybir.AluOpType

    with TileContext(nc) as tc:
        with (
            tc.tile_pool(name="weights", bufs=1) as pw,
            tc.tile_pool(name="xbuf", bufs=4) as px,
            tc.tile_pool(name="signs", bufs=3) as psn,
            tc.tile_pool(name="small", bufs=4) as pt,
            tc.tile_pool(name="psA", bufs=4, space="PSUM") as psum1,
            tc.tile_pool(name="psB", bufs=4, space="PSUM") as psum2,
        ):
            w1_t = pw.tile([128, 36 * 128], BF16, tag="w1")
            nc.sync.dma_start(out=w1_t[:, :],
                              in_=w1_ext[:, :, :].rearrange("i a o -> i (a o)"))
            w2_t = pw.tile([128, 18 * 128], BF16, tag="w2")
            nc.sync.dma_start(out=w2_t[:, :],
                              in_=w2_ext[:, :, :].rearrange("i a o -> i (a o)"))
            coef_t = pw.tile([128, 10], F32, tag="coef")
            nc.sync.dma_start(out=coef_t[:, :], in_=coef_ext[:, :])

            st = [dict() for _ in range(BL)]

            def sign_stage(i, src_tiles, bias_col, tag):
                s = psn.tile([128, 2 * PS], BF16, tag="s")
                sv = s[:, :].rearrange("p (h y x) -> p h y x", h=2, y=PH, x=PW)
                for h in range(2):
                    nc.vector.memset(sv[:, h, 0, :], 0)
                    nc.vector.memset(sv[:, h, PH - 1, :], 0)
                    nc.vector.memset(sv[:, h, 1:PH - 1, 0], 0)
                    nc.vector.memset(sv[:, h, 1:PH - 1, PW - 1], 0)
                    nc.scalar.sign(
                        sv[:, h, 1:1 + H, 1:1 + W],
                        src_tiles[h][:, :].rearrange("p (y x) -> p y x", y=H, x=W),
                        bias=coef_t[:, bias_col + h:bias_col + h + 1],
                    )
                st[i][tag] = sv

            def stage_A(i):
                xs = []
                for h in range(2):
                    xt = px.tile([128, HW_], F32, tag="x")
                    nc.sync.dma_start(
                        out=xt[:, :],
                        in_=x_ext[i, h * 128:(h + 1) * 128, :, :].rearrange(
                            "c y x -> c (y x)"),
                    )
                    xs.append(xt)
                st[i]["x"] = xs
                sign_stage(i, xs, 2, "s1")

            def stage_B(i):  # conv1 + xres (in place into x)
                sv = st[i]["s1"]
                xs = st[i]["x"]
                for g in range(2):
                    for bp in range(4):          # pairs of 8-row blocks
                        pts = [psum1.tile([128, 512], F32, tag="ps1", name=f"p1_{i}_{g}_{bp}_{q}")
                               for q in range(2)]
                        n = 0
                        for t in range(9):
                            ky, kx = t // 3, t % 3
                            for h in range(2):
                                idx = (g * 2 + h) * 9 + t
                                wap = w1_t[:, idx * 128:(idx + 1) * 128]
                                for half in range(2):
                                    y0 = (bp * 2 + half) * 8
                                    nc.tensor.matmul(
                                        pts[half][:, :], wap,
                                        sv[:, h, y0 + ky:y0 + ky + 8, kx:kx + W],
                                        start=(n == 0), stop=(n == 17),
                                    )
                                n += 1
                        for half in range(2):
                            blk = bp * 2 + half
                            t1 = pt.tile([128, 512], F32, tag="t1")
                            nc.scalar.mul(t1[:, :], pts[half][:, :],
                                          coef_t[:, g:g + 1])
                            xg = xs[g][:, blk * 512:(blk + 1) * 512]
                            nc.vector.tensor_add(xg, xg, t1[:, :])

            def stage_C(i):
                sign_stage(i, st[i]["x"], 4, "s2")

            def stage_D(i):  # conv2 + epilogue + out DMA
                sv = st[i]["s2"]
                x0 = st[i]["x"][0]
                for bp in range(4):
                    pts = [psum2.tile([128, 512], F32, tag="ps2", name=f"p2_{i}_{bp}_{q}")
                           for q in range(2)]
                    n = 0
                    for t in range(9):
                        ky, kx = t // 3, t % 3
                        for h in range(2):
                            idx = h * 9 + t
                            wap = w2_t[:, idx * 128:(idx + 1) * 128]
                            for half in range(2):
                                y0 = (bp * 2 + half) * 8
                                nc.tensor.matmul(
                                    pts[half][:, :], wap,
                                    sv[:, h, y0 + ky:y0 + ky + 8, kx:kx + W],
                                    start=(n == 0), stop=(n == 17),
                                )
                            n += 1
                    for half in range(2):
                        blk = bp * 2 + half
                        pt2 = pts[half]
                        t2 = pt.tile([128, 512], F32, tag="t1")
                        nc.scalar.activation(t2[:, :], pt2[:, :], Ident,
                                             bias=coef_t[:, 7:8],
                                             scale=coef_t[:, 6:7])
                        xb = x0[:, blk * 512:(blk + 1) * 512]
                        nc.vector.tensor_add(xb, xb, t2[:, :])   # u = t2 + xres
                        m = pt.tile([128, 512], F32, tag="m")
                        nc.vector.tensor_scalar(m[:, :], xb, 0.0,
                                                coef_t[:, 8:9],
                                                op0=Alu.min, op1=Alu.mult)
                        nc.vector.tensor_add(xb, xb, m[:, :])    # u = prelu(u)
                uv = x0[:, :].rearrange("p (h2 r1 w2 r2) -> p r1 r2 h2 w2",
                                        h2=32, r1=2, w2=32, r2=2)
                od = out_ext[i, :, :, :].rearrange("(c j) y x -> c j y x", j=4)
                for j in range(4):
                    r1, r2 = j >> 1, j & 1
                    y = pt.tile([128, 1024], F32, tag="y")
                    nc.scalar.activation(
                        y[:, :].rearrange("p (a b) -> p a b", a=32, b=32),
                        uv[:, r1, r2, :, :], Ident,
                        bias=coef_t[:, 9:10], scale=1.0)
                    nc.sync.dma_start(
                        out=od[:, j, :, :],
                        in_=y[:, :].rearrange("p (a b) -> p a b", a=32, b=32))

            # software-pipelined emission: keep the PE busy across images
            for _ in range(reps):
                stage_A(0)
                stage_A(1)
                stage_B(0)
                stage_C(0)
                stage_B(1)
                stage_C(1)
                stage_D(0)
                stage_D(1)

    nc.compile()
    return nc


def _prep_weights(inputs):
    w1 = np.asarray(inputs["conv1_w"], np.float32)          # [256,256,3,3]
    w2 = np.asarray(inputs["conv2_w"], np.float32)          # [128,256,3,3]
    sc1 = (np.abs(w1).mean(axis=(1, 2, 3))
           * float(np.asarray(inputs["kw1"]))
           * float(np.asarray(inputs["ka1"]))).astype(np.float32)   # [256]
    sc2 = (np.abs(w2).mean(axis=(1, 2, 3))
           * float(np.asarray(inputs["kw2"]))
           * float(np.asarray(inputs["ka2"]))).astype(np.float32)   # [128]

    sgn1 = np.sign(w1).reshape(2, 128, 2, 128, 9)           # [g,o,h,i,t]
    w1b = np.ascontiguousarray(sgn1.transpose(3, 0, 2, 4, 1)
                               ).reshape(128, 36, 128).astype(ml_dtypes.bfloat16)
    sgn2 = np.sign(w2).reshape(128, 2, 128, 9)              # [o,h,i,t]
    w2b = np.ascontiguousarray(sgn2.transpose(2, 1, 3, 0)
                               ).reshape(128, 18, 128).astype(ml_dtypes.bfloat16)

    coef = np.zeros((128, 10), np.float32)
    coef[:, 0] = sc1[:128]
    coef[:, 1] = sc1[128:]
    b1 = np.asarray(inputs["bias1_"], np.float32).reshape(C)
    coef[:, 2] = b1[:128]
    coef[:, 3] = b1[128:]
    b2 = np.asarray(inputs["bias2_"], np.float32).reshape(C)
    coef[:, 4] = b2[:128]
    coef[:, 5] = b2[128:]
    coef[:, 6] = sc2
    coef[:, 7] = np.asarray(inputs["bias3"], np.float32).reshape(C // 2)
    coef[:, 8] = np.asarray(inputs["prelu2_w"], np.float32) - 1.0
    coef[:, 9] = np.asarray(inputs["bias4"], np.float32).reshape(C // 2)
    return w1b, w2b, coef


def kernel(**inputs):
    return kernel_with_results(**inputs)[0]


def kernel_with_results(trace=False, **inputs):
    x = np.ascontiguousarray(np.asarray(inputs["x"], np.float32))
    w1b, w2b, coef = _prep_weights(inputs)

    if "nc" not in _CACHE:
        _CACHE["nc"] = build_nc()
    nc = _CACHE["nc"]

    in_maps = [
        {"x": x[i * BL:(i + 1) * BL], "w1": w1b, "w2": w2b, "coef": coef}
        for i in range(NCORES)
    ]
    res = run_bass_kernel_spmd(nc, in_maps, core_ids=list(range(NCORES)),
                               trace=trace)
    out = np.concatenate([res.results[i]["out"] for i in range(NCORES)], axis=0)
    return out, res


# revision 11
# speedup vs baseline: 3.3019x; 1.5157x over previous
"""AdaBlock (binarized double-conv residual block) Trainium2 kernel.

Strategy
--------
Data-parallel over batch: 16 images across 8 NeuronCores (2 images/core), no
collectives.  The binarized convs are exact +-1 matmuls: a 3x3 conv is 9
shifted [Cin x spatial] matmuls accumulated in PSUM.  Signs are exact in
fp8e4m3, and `perf_mode=DoubleRow` packs both 128-channel cin halves into one
K=256 matmul at 2x the bf16 rate.  Per-out-channel scales (ka * mean|w| * kw)
are applied while draining PSUM; results match the fp32 reference to ~1e-6.

Spatial layout: sign activations live in a zero-ring-padded 66x66 grid per
cin half (flat, half-stride 4368 so the DoubleRow rhs AP is [p, 2, N]).
Conv output is tiled over 7 padded rows (N=462) per PSUM bank plus a 1-row
runt, so every drain is a single strided op into the flat 64x64 layout.

Per-core pipeline (per image):
  DMA x -> s1 = sign(x + bias1_)           (fp8, padded grid)
  conv1: 2 outgrps x 10 row-tiles x 9 taps DoubleRow matmuls -> PSUM
  xres  = psum * sc1 + x                   (in-place in the x buffer)
  s2    = sign(xres + bias2_)
  conv2: 10 row-tiles x 9 taps -> PSUM
  u     = psum * sc2 + bias3 + xres[:128];  u = prelu(u)  (in-place)
  pixel-unshuffle via 4 strided copies fused with +bias4 -> DMA out
"""

import numpy as np
import ml_dtypes

import concourse.bass as bass
import concourse.mybir as mybir
from concourse import bacc
from concourse.tile import TileContext
from concourse.bass_utils import run_bass_kernel_spmd

B, C, H, W = 16, 256, 64, 64
NCORES = 8
BL = B // NCORES          # images per core
HW_ = H * W               # 4096
PW = W + 2                # 66 padded row width
HS = 4368                 # per-half stride in the sign buffer (16-aligned)
GRID = 1                  # padded 66x66 grid starts at this offset in a half
F32 = mybir.dt.float32
FP8 = mybir.dt.float8e4
DR = mybir.MatmulPerfMode.DoubleRow

# row-tiles: 9 tiles of 7 output rows + 1 runt row
TILES = [(t * 7, 7) for t in range(9)] + [(63, 1)]

_CACHE = {}


def build_nc(reps=1):
    nc = bacc.Bacc()
    x_ext = nc.declare_dram_parameter("x", [BL, C, H, W], F32, isOutput=False)
    w1_ext = nc.declare_dram_parameter("w1", [128, 18 * 256], FP8, isOutput=False)
    w2_ext = nc.declare_dram_parameter("w2", [128, 9 * 256], FP8, isOutput=False)
    coef_ext = nc.declare_dram_parameter("coef", [128, 10], F32, isOutput=False)
    out_ext = nc.declare_dram_parameter("out", [BL, 2 * C, H // 2, W // 2], F32,
                                        isOutput=True)

    Ident = mybir.ActivationFunctionType.Identity
    Alu = mybir.AluOpType

    with TileContext(nc) as tc:
        with (
            tc.tile_pool(name="weights", bufs=1) as pw,
            tc.tile_pool(name="xbuf", bufs=4) as px,
            tc.tile_pool(name="signs", bufs=3) as psn,
            tc.tile_pool(name="small", bufs=4) as pt,
            tc.tile_pool(name="ps", bufs=8, space="PSUM") as psum,
        ):
            w1_t = pw.tile([128, 18 * 256], FP8, tag="w1")
            nc.sync.dma_start(out=w1_t[:, :], in_=w1_ext[:, :])
            w2_t = pw.tile([128, 9 * 256], FP8, tag="w2")
            nc.sync.dma_start(out=w2_t[:, :], in_=w2_ext[:, :])
            coef_t = pw.tile([128, 10], F32, tag="coef")
            nc.sync.dma_start(out=coef_t[:, :], in_=coef_ext[:, :])

            st = [dict() for _ in range(BL)]

            def sign_stage(i, src_tiles, bias_col, tag):
                s = psn.tile([128, 2 * HS], FP8, tag="s", name=f"s_{tag}_{i}")
                sv = s[:, :].rearrange("p (h q) -> p h q", h=2, q=HS)
                for h in range(2):
                    # zero the pad ring + margins of the 66x66 grid
                    nc.vector.memset(sv[:, h, 0:GRID + PW + 1], 0)
                    nc.vector.memset(sv[:, h, GRID + 65 * PW:HS], 0)
                    lc = sv[:, h, GRID + PW:GRID + PW + 64 * PW].rearrange(
                        "p (r c) -> p r c", c=PW)
                    nc.vector.memset(lc[:, :, 0], 0)
                    nc.vector.memset(lc[:, :, PW - 1], 0)
                    # interior: s = sign(src + bias), cast to fp8
                    nc.scalar.sign(
                        sv[:, h, GRID + PW:GRID + PW + 64 * PW].rearrange(
                            "p (r c) -> p r c", c=PW)[:, :, 1:1 + W],
                        src_tiles[h][:, :].rearrange("p (y x) -> p y x", y=H),
                        bias=coef_t[:, bias_col + h:bias_col + h + 1],
                    )
                st[i][tag] = sv

            def stage_A(i):
                xs = []
                for h in range(2):
                    xt = px.tile([128, HW_], F32, tag="x", name=f"x_{i}_{h}")
                    nc.sync.dma_start(
                        out=xt[:, :],
                        in_=x_ext[i, h * 128:(h + 1) * 128, :, :].rearrange(
                            "c y x -> c (y x)"),
                    )
                    xs.append(xt)
                st[i]["x"] = xs
                sign_stage(i, xs, 2, "s1")

            def conv(i, sv, w_t, ngrp, drain):
                # tiles grouped 4/4/2 so each stationary weight feeds 4 MMs
                for g in range(ngrp):
                    for tb in (TILES[0:4], TILES[4:8], TILES[8:10]):
                        pts = []
                        for q, (y0, rows) in enumerate(tb):
                            pts.append(psum.tile([128, 512], F32, tag="ps",
                                                 name=f"ps_{i}_{g}_{y0}"))
                        for t in range(9):
                            ky, kx = t // 3, t % 3
                            wap = w_t[:, (g * 9 + t) * 256:(g * 9 + t + 1) * 256
                                      ].rearrange("p (h m) -> p h m", h=2)
                            for q, (y0, rows) in enumerate(tb):
                                n = rows * PW
                                off = GRID + PW * (y0 + ky) + kx - 1
                                nc.tensor.matmul(
                                    pts[q][:, :n], wap,
                                    sv[:, :, off:off + n],
                                    start=(t == 0), stop=(t == 8),
                                    perf_mode=DR,
                                )
                        for q, (y0, rows) in enumerate(tb):
                            drain(g, y0, rows, pts[q])

            def stage_B(i):  # conv1 + xres (in place into x)
                xs = st[i]["x"]

                def drain(g, y0, rows, ps):
                    n = rows * W
                    t1 = pt.tile([128, 448], F32, tag="t1")
                    src = ps[:, :rows * PW].rearrange(
                        "p (r c) -> p r c", c=PW)[:, :, 1:1 + W]
                    nc.scalar.mul(
                        t1[:, :n].rearrange("p (r c) -> p r c", c=W),
                        src, coef_t[:, g:g + 1])
                    xg = xs[g][:, y0 * W:y0 * W + n]
                    nc.vector.tensor_add(xg, xg, t1[:, :n])

                conv(i, st[i]["s1"], w1_t, 2, drain)

            def stage_C(i):
                sign_stage(i, st[i]["x"], 4, "s2")

            def stage_D(i):  # conv2 + epilogue + out DMA
                x0 = st[i]["x"][0]

                def drain(g, y0, rows, ps):
                    n = rows * W
                    t2 = pt.tile([128, 448], F32, tag="t1")
                    src = ps[:, :rows * PW].rearrange(
                        "p (r c) -> p r c", c=PW)[:, :, 1:1 + W]
                    nc.scalar.activation(
                        t2[:, :n].rearrange("p (r c) -> p r c", c=W),
                        src, Ident, bias=coef_t[:, 7:8], scale=coef_t[:, 6:7])
                    xb = x0[:, y0 * W:y0 * W + n]
                    nc.vector.tensor_add(xb, xb, t2[:, :n])   # u = t2 + xres
                    m = pt.tile([128, 448], F32, tag="m")
                    nc.vector.tensor_scalar(m[:, :n], xb, 0.0, coef_t[:, 8:9],
                                            op0=Alu.min, op1=Alu.mult)
                    nc.vector.tensor_add(xb, xb, m[:, :n])    # u = prelu(u)

                conv(i, st[i]["s2"], w2_t, 1, drain)

                uv = x0[:, :].rearrange("p (h2 r1 w2 r2) -> p r1 r2 h2 w2",
                                        h2=32, r1=2, w2=32, r2=2)
                od = out_ext[i, :, :, :].rearrange("(c j) y x -> c j y x", j=4)
                for j in range(4):
                    r1, r2 = j >> 1, j & 1
                    y = pt.tile([128, 1024], F32, tag="y")
                    nc.scalar.activation(
                        y[:, :].rearrange("p (a b) -> p a b", a=32, b=32),
                        uv[:, r1, r2, :, :], Ident,
                        bias=coef_t[:, 9:10], scale=1.0)
                    nc.sync.dma_start(
                        out=od[:, j, :, :],
                        in_=y[:, :].rearrange("p (a b) -> p a b", a=32, b=32))

            # software-pipelined emission: keep the PE busy across images
            for _ in range(reps):
                stage_A(0)
                stage_A(1)
                stage_B(0)
                stage_C(0)
                stage_B(1)
                stage_C(1)
                stage_D(0)
                stage_D(1)

    nc.compile()
    return nc


def _prep_weights(inputs):
    w1 = np.asarray(inputs["conv1_w"], np.float32)          # [256,256,3,3]
    w2 = np.asarray(inputs["conv2_w"], np.float32)          # [128,256,3,3]
    sc1 = (np.abs(w1).mean(axis=(1, 2, 3))
           * float(np.asarray(inputs["kw1"]))
           * float(np.asarray(inputs["ka1"]))).astype(np.float32)   # [256]
    sc2 = (np.abs(w2).mean(axis=(1, 2, 3))
           * float(np.asarray(inputs["kw2"]))
           * float(np.asarray(inputs["ka2"]))).astype(np.float32)   # [128]

    # w1b[i, g, t, h, o] = sign(w1)[g*128+o, h*128+i, t//3, t%3]
    sgn1 = np.sign(w1).reshape(2, 128, 2, 128, 9)           # [g,o,h,i,t]
    w1b = np.ascontiguousarray(sgn1.transpose(3, 0, 4, 2, 1)
                               ).reshape(128, 18 * 256).astype(
                                   ml_dtypes.float8_e4m3fn)
    sgn2 = np.sign(w2).reshape(128, 2, 128, 9)              # [o,h,i,t]
    w2b = np.ascontiguousarray(sgn2.transpose(2, 3, 1, 0)
                               ).reshape(128, 9 * 256).astype(
                                   ml_dtypes.float8_e4m3fn)

    coef = np.zeros((128, 10), np.float32)
    coef[:, 0] = sc1[:128]
    coef[:, 1] = sc1[128:]
    b1 = np.asarray(inputs["bias1_"], np.float32).reshape(C)
    coef[:, 2] = b1[:128]
    coef[:, 3] = b1[128:]
    b2 = np.asarray(inputs["bias2_"], np.float32).reshape(C)
    coef[:, 4] = b2[:128]
    coef[:, 5] = b2[128:]
    coef[:, 6] = sc2
    coef[:, 7] = np.asarray(inputs["bias3"], np.float32).reshape(C // 2)
    coef[:, 8] = np.asarray(inputs["prelu2_w"], np.float32) - 1.0
    coef[:, 9] = np.asarray(inputs["bias4"], np.float32).reshape(C // 2)
    return w1b, w2b, coef


def kernel(**inputs):
    return kernel_with_results(**inputs)[0]


def kernel_with_results(trace=False, **inputs):
    x = np.ascontiguousarray(np.asarray(inputs["x"], np.float32))
    w1b, w2b, coef = _prep_weights(inputs)

    if "nc" not in _CACHE:
        _CACHE["nc"] = build_nc()
    nc = _CACHE["nc"]

    in_maps = [
        {"x": x[i * BL:(i + 1) * BL], "w1": w1b, "w2": w2b, "coef": coef}
        for i in range(NCORES)
    ]
    res = run_bass_kernel_spmd(nc, in_maps, core_ids=list(range(NCORES)),
                               trace=trace)
    out = np.concatenate([res.results[i]["out"] for i in range(NCORES)], axis=0)
    return out, res
